# revision 1
# baseline (speedup 1.0000x reference)
"""Trainium2 Bass kernel for attention-weighted pooling.

Computes, for x[B,T,D], W[D,1], b[T,1]:
    et = tanh(x @ W + b)            # (B, T)
    at = softmax(et, axis=-1)       # (B, T)
    out = einsum('btd,bt->bd', x, at)

Sharding: pure data parallel over batch across 8 NeuronCores (4 batches per
core); W and b replicated. No collectives.

Key structure (per core, streaming single pass over x):
  - tanh output is bounded in [-1, 1], so softmax needs no max subtraction;
    exp() cannot overflow. Normalization by the denominator is deferred to
    the very end, so x is read from HBM exactly once (memory roofline;
    measured stream ~424 GB/s read, ~83.3us for the 32 MiB/core).
  - x is cast fp32 -> fp16 during the DMA itself (SWDGE cast): HBM traffic
    stays the required 32 MiB/core of fp32, but on-chip x is half the bytes.
    This makes the PE matmul single-pass (fp32 matmul lowers to an HI/LO
    pass PAIR - always above the DMA roofline) and makes the DVE ops
    eligible for the 2x fp16 mode.
  - ALL 16 x chunks (2 MiB each) stay resident in SBUF (128 KiB/partition)
    so the DMA stream is never throttled by buffer reuse.
  - Per 2-MiB chunk [128 x 8 x 512]: the 8 row-dot-products elin[t] =
    sum_d x[t,d]*W[d] are split between engines so both stay under the DMA
    pace (~677ns/subtile): 4 via fused DVE scalar_tensor_tensor (1x,
    ~690ns), 4 via DVE tensor_mul (2x fp16, ~365ns) + ACT Copy with
    accum_out (~1.0us incl accumulator read). DVE adds b; ACT does tanh
    then exp; PE accumulates p.T @ x_subtile into PSUM [1, D] (~554ns).
  - Per-batch epilogue: S = sum_t p_t (free-dim reduce + ones-matmul),
    out_row = acc / S into a [1, 4*D] staging tile; one 8 KiB output DMA.
  - Pair-row DMA layout: each descriptor covers 2 adjacent t-rows (4 KiB
    contiguous HBM) halving per-descriptor overhead vs row-per-partition
    (measured 414 -> 424 GB/s; 8 KiB descriptors regress, packets cap at 4K).
  - Measured ~108.4us. Do NOT: put compute on the gpsimd queue (it gates the
    SWDGE DMA emissions, +30us), use vector.tensor_tensor_reduce (hardware
    crash), or taper chunk sizes (adds per-chunk overhead on the binding
    engines). See memory notes for the full failed-experiment list.
"""

import sys

sys.path.insert(0, "/opt/trn_rl_repo")

import numpy as np

B, T, D = 32, 4096, 512
N_CORES = 8
B_LOCAL = B // N_CORES          # 4 batches per core
P = 128                         # SBUF partitions
TS_T = 1024                     # t-rows per super-tile (2 MiB fp32 DMA)
N_ST = T // TS_T                # 4 super-tiles per batch
N_J = TS_T // P                 # 8 t-subtiles per super-tile
N_STT = 4                       # subtiles 0..N_STT-1 use the fused DVE op;
                                # the rest use DVE mult + ACT accum-reduce

_PROGRAM = None


def _build_program():
    import concourse.bacc as bacc
    import concourse.bass_isa as bass_isa
    import concourse.mybir as mybir
    import concourse.tile as tile

    f32 = mybir.dt.float32
    bf16 = mybir.dt.float16
    nc = bacc.Bacc("TRN2", target_bir_lowering=False, debug=False)

    x_d = nc.dram_tensor("x", [B_LOCAL, T, D], f32, kind="ExternalInput")
    W_d = nc.dram_tensor("W", [D, 1], f32, kind="ExternalInput")
    b_d = nc.dram_tensor("b", [T, 1], f32, kind="ExternalInput")
    o_d = nc.dram_tensor("out", [B_LOCAL, D], f32, kind="ExternalOutput")

    with tile.TileContext(nc) as tc:
        with (
            tc.tile_pool(name="consts", bufs=1) as consts,
            # All 16 x chunks stay resident (128 KiB/partition) so the DMA
            # stream is never throttled by SBUF slot reuse.
            tc.tile_pool(name="xin", bufs=16) as xin,
            tc.tile_pool(name="scratch", bufs=4) as scratch_pool,
            tc.tile_pool(name="prod", bufs=6) as prod_pool,
            tc.tile_pool(name="small", bufs=8) as small,
            tc.tile_pool(name="pbuf", bufs=2) as pbuf_pool,
            tc.tile_pool(name="acc_psum", bufs=2, space="PSUM") as acc_psum_pool,
            tc.tile_pool(name="s_psum", bufs=2, space="PSUM") as s_psum_pool,
        ):
            # Per-batch tile plan: (t0, n_j) chunks, all full 2-MiB tiles.
            # (Head/tail chunk tapering measured neutral-to-worse: the Tile
            # scheduler re-converges to its own pipeline depth regardless of
            # start time, and tail tapering adds per-chunk ACT overhead
            # exactly where the engines are binding.)
            full = [(t0, N_J) for t0 in range(0, T, TS_T)]
            plans = [full] * B_LOCAL
            n_bufs = {8: 16}

            def issue_x_dma(bb, t0, nj):
                # Pair-row layout: subtile column c = jj*2+i holds row
                # t = t0 + jj*256 + 2p + i on partition p, so each DMA
                # descriptor covers TWO adjacent t-rows = 4 KiB contiguous
                # HBM (half the per-descriptor overhead of the row-per-
                # partition layout). Softmax/pooling are permutation-
                # invariant over t; only b_buf must match this layout.
                xt = xin.tile(
                    [P, nj // 2, 2, D], bf16, tag=f"xt{nj}", bufs=n_bufs[nj],
                    name="xt",
                )
                nc.gpsimd.dma_start(
                    xt[:],
                    x_d.ap()[bb, t0 : t0 + nj * P, :].rearrange(
                        "(jj p i) d -> p jj i d", p=P, i=2
                    ),
                )
                return xt

            # Pre-issue the first x chunk so the SWDGE queue starts streaming
            # x before the W broadcast.
            pre = {(0, 0): issue_x_dma(0, *plans[0][0])}

            # W broadcast to all 128 partitions, cast to bf16: [128, D].
            # (Loading W via sync + PE ones-matmul broadcast instead measured
            # WORSE on a clean run — drain grew 9.8 -> 13.4us — keep the
            # simple SWDGE broadcast DMA.)
            w_bcast = consts.tile([P, D], bf16)
            nc.gpsimd.dma_start(
                w_bcast[:],
                W_d.ap().rearrange("d one -> one d").broadcast_to([P, D]),
            )
            # b laid out to match the pair-row x layout
            # t = st*TS_T + jj*256 + 2p + i: [128, N_ST, N_J/2, 2]
            b_buf = consts.tile([P, N_ST, N_J // 2, 2], f32)
            nc.sync.dma_start(
                b_buf[:],
                b_d.ap().rearrange(
                    "(st jj p i) one -> p st jj (i one)",
                    st=N_ST, jj=N_J // 2, p=P, i=2,
                ),
            )
            ones_col = consts.tile([P, 1], f32)
            nc.vector.memset(ones_col[:], 1.0)
            # All batch outputs gathered on one partition; single 8 KiB
            # contiguous DMA at the end.
            out_all = consts.tile([1, B_LOCAL * D], f32)

            for bb in range(B_LOCAL):
                p_buf = pbuf_pool.tile([P, T // P], bf16)
                acc = acc_psum_pool.tile([1, D], f32)

                chunks = plans[bb]
                total_mm = sum(nj for _, nj in chunks)
                mm_idx = 0
                for ci, (t0, nj) in enumerate(chunks):
                    col0 = t0 // P
                    # SWDGE dma with inline fp32->fp16 cast
                    xt = pre.pop((bb, ci), None)
                    if xt is None:
                        xt = issue_x_dma(bb, t0, nj)
                    elin = small.tile([P, nj], f32)
                    # Half the subtiles: fused mult+reduce on DVE
                    # (scalar_tensor_tensor, 1x uop ~690ns). Other half: plain
                    # tensor_tensor mult on DVE (fp16 2x_1P mode, ~360ns) with
                    # the reduce offloaded to ACT (activation Copy +
                    # accum_out, ~1.0us incl accumulator read). This splits
                    # the per-element dot-product work so both engines stay
                    # under the DMA roofline. (Keep the GPSIMD queue DMA-only:
                    # compute placed there gates SWDGE emissions.) Tiny head
                    # chunks take the all-fused path: one DVE op has much
                    # lower serial latency than mult->ACT->accum.
                    n_stt = nj if nj <= 2 else nj // 2
                    for j in range(n_stt):
                        scratch = scratch_pool.tile([P, D], bf16)
                        nc.vector.scalar_tensor_tensor(
                            out=scratch[:],
                            in0=xt[:, j >> 1, j & 1, :],
                            scalar=1.0,
                            in1=w_bcast[:],
                            op0=mybir.AluOpType.mult,
                            op1=mybir.AluOpType.mult,
                            accum_out=elin[:, j : j + 1],
                        )
                    for j in range(n_stt, nj):
                        prod = prod_pool.tile([P, D], bf16)
                        nc.vector.tensor_mul(prod[:], xt[:, j >> 1, j & 1, :], w_bcast[:])
                        nc.scalar.activation(
                            prod[:],
                            prod[:],
                            mybir.ActivationFunctionType.Copy,
                            accum_out=elin[:, j : j + 1],
                        )
                    for ws, wn in ((0, nj),):
                        ee = small.tile([P, wn], f32)
                        nc.vector.tensor_add(
                            ee[:],
                            elin[:, ws : ws + wn],
                            b_buf[:, t0 // TS_T, :, :].rearrange(
                                "p a b -> p (a b)"
                            ),
                        )
                        et = small.tile([P, wn], f32)
                        nc.scalar.activation(
                            et[:], ee[:], mybir.ActivationFunctionType.Tanh
                        )
                        nc.scalar.activation(
                            p_buf[:, col0 + ws : col0 + ws + wn],
                            et[:],
                            mybir.ActivationFunctionType.Exp,
                        )
                        for j in range(ws, ws + wn):
                            nc.tensor.matmul(
                                acc[:],
                                p_buf[:, col0 + j : col0 + j + 1],
                                xt[:, j >> 1, j & 1, :],
                                start=(mm_idx == 0),
                                stop=(mm_idx == total_mm - 1),
                            )
                            mm_idx += 1

                # denominator S = sum_t p_t  (free-dim reduce, then
                # cross-partition reduce via ones-matmul)
                ssum = small.tile([P, 1], f32)
                nc.vector.reduce_sum(ssum[:], p_buf[:], axis=mybir.AxisListType.X)
                if bb < B_LOCAL - 1:
                    # Mid-stream batches: cross-partition sum via ones-matmul
                    # (PE queue), final scale on ACT — both overlap the
                    # ongoing stream.
                    s_ps = s_psum_pool.tile([1, 1], f32)
                    nc.tensor.matmul(s_ps[:], ssum[:], ones_col[:])
                    sinv = small.tile([1, 1], f32)
                    nc.vector.reciprocal(sinv[:], s_ps[:])
                    nc.scalar.mul(
                        out_all[:, bb * D : (bb + 1) * D], acc[:], sinv[:]
                    )
                else:
                    # Last batch: its epilogue is the exposed tail. The PE
                    # queue is still draining pooling matmuls, so compute S
                    # on GPSIMD instead (its queue is empty once the last x
                    # emission is out) — S/recip are then ready before the
                    # last matmul retires, and the final scale runs on DVE
                    # right after its own reciprocal (no cross-engine hop).
                    s_all = small.tile([P, 1], f32)
                    nc.gpsimd.partition_all_reduce(
                        s_all[:], ssum[:], channels=P,
                        reduce_op=bass_isa.ReduceOp.add,
                    )
                    sinv = small.tile([1, 1], f32)
                    nc.vector.reciprocal(sinv[:], s_all[0:1, :])
                    nc.vector.tensor_scalar_mul(
                        out_all[:, bb * D : (bb + 1) * D], acc[:], sinv[:]
                    )

            nc.sync.dma_start(
                o_d.ap().rearrange("(one b) d -> one (b d)", one=1), out_all[:]
            )

    nc.compile()
    return nc


def _get_program():
    global _PROGRAM
    if _PROGRAM is None:
        _PROGRAM = _build_program()
    return _PROGRAM


def _shard_inputs(x, W, b):
    x = np.ascontiguousarray(np.asarray(x, dtype=np.float32))
    W = np.ascontiguousarray(np.asarray(W, dtype=np.float32))
    b = np.ascontiguousarray(np.asarray(b, dtype=np.float32))
    return [
        {"x": x[c * B_LOCAL : (c + 1) * B_LOCAL], "W": W, "b": b}
        for c in range(N_CORES)
    ]


def _install_ntff_hook_shim():
    """The agent image's ``antenv`` lacks ``axon_hooks``, so the boot-time
    NTFF hook registration silently degrades. Recreate the module in
    sys.modules and register the ctypes hook against libaxon_pjrt.so."""
    import types

    if "antenv.axon_hooks" in sys.modules:
        return
    mod = types.ModuleType("antenv.axon_hooks")
    _hook = [None]
    mod.set_axon_ntff_profile_hook = lambda h: _hook.__setitem__(0, h)
    mod.get_axon_ntff_profile_hook = lambda: _hook[0]
    import antenv

    antenv.axon_hooks = mod
    sys.modules["antenv.axon_hooks"] = mod
    try:
        sys.path.insert(0, "/root/.axon_site")
        from trn_agent_boot.trn_boot import _ntff_profile_via_ctypes

        mod.set_axon_ntff_profile_hook(
            _ntff_profile_via_ctypes("/opt/axon/libaxon_pjrt.so")
        )
    except Exception as e:  # profiling is best-effort; run still works
        print(f"NTFF hook shim failed ({e}); tracing disabled", file=sys.stderr)


def _run(in_maps, trace=False):
    from concourse.bass_utils import run_bass_kernel_spmd

    nc = _get_program()
    kwargs = {}
    if trace:
        _install_ntff_hook_shim()
        kwargs = {"trace": True, "trace_cores": [0]}
    return run_bass_kernel_spmd(nc, in_maps, core_ids=list(range(N_CORES)), **kwargs)


def kernel(x, W, b):
    res = _run(_shard_inputs(x, W, b))
    return np.concatenate(
        [res.results[c]["out"] for c in range(N_CORES)], axis=0
    ).astype(np.float32)


def kernel_profiled(x, W, b):
    """Like kernel() but also returns the NTFF-measured exec time in ns."""
    res = _run(_shard_inputs(x, W, b), trace=True)
    out = np.concatenate(
        [res.results[c]["out"] for c in range(N_CORES)], axis=0
    ).astype(np.float32)
    return out, res

